# revision 1
# baseline (speedup 1.0000x reference)
"""Trainium2 Bass kernel for nn_Architecture_54451595379019 (ConvGRU top-down
message passing net, N=4 nodes, B=32, 2 reps).

Strategy (8 cores):
  * Structural simplification (exact): initial states are zero, so in rep 0
    only node 0 sees batch data; nodes 1..3 rep-0 states are batch-constant
    (functions of biases only).  In rep 1 the top-down GEMM inputs are those
    batch-constant states -> the 3 big td GEMMs (6144x4096) become GEMVs.
    bu GEMM for node 0 is identical in both reps (input = conv_in(x)).
  * td GEMVs: output-dim sharded 8 ways (768 of 6144 per core) + AllGather.
  * bu GEMMs: contraction(k)-sharded 8 ways ((512,2048) slice of bu_w[n].T
    per core), partials ReduceScatter-ed over the batch axis.
  * cells (3x3 convs + gates): batch-sharded, 4 images/core, convs as 9
    shifted-tap matmuls on zero-padded SBUF buffers.
  * states exchanged with AllToAll -> each core receives the batch-major
    [32, 512] k-slice for the next GEMM (pure SPMD; per-core data differs
    only in input tensor values).
  * fp32 storage/DMA; bf16 matmul operands (cast in SWDGE DMA or on DVE
    write); fp32 PSUM accumulation and fp32 elementwise chain.

HW constraints honored: compute-op partition base must be 32-aligned (we use
base 0 everywhere); both-SBUF-input ops need equal bases; DMA is exempt, so
all writes into partitions [8:24) of conv input buffers and the extraction
of the u-gate (rows 16:32) go through DMA.
"""

import numpy as np

import concourse.bass as bass
import concourse.bacc as bacc
import concourse.mybir as mybir
import concourse.tile as tile
from concourse.bass_utils import run_bass_kernel_spmd
from concourse.masks import make_identity

F32 = mybir.dt.float32
BF16 = mybir.dt.bfloat16
AOP = mybir.AluOpType
ACT = mybir.ActivationFunctionType

R = 8          # cores
B = 32         # batch
BL = B // R    # images per core
NND = 4        # nodes
HD = 16        # hidden channels
CIN = 8        # input channels
TD_C = CIN + HD
HW = 256       # 16*16 spatial
KF = HD * HW   # 4096 hidden-flat (GEMM contraction)
KS = KF // R   # 512 contraction slice per core
JBU = CIN * HW         # 2048 bu output
JTD = TD_C * HW // R   # 768 td output slice per core
TAPS = [(dy, dx) for dy in range(3) for dx in range(3)]

_CACHED_NC = None
import os
TRUNC = int(os.environ.get("KTRUNC", "99"))


def _conv9(nc, ps_pool, writer, zpad_v, w_sb, oc, nimg):
    """3x3 SAME conv as 9 shifted-tap matmuls accumulating in PSUM.

    zpad_v: SBUF view [ic, nimg, 18, 18] bf16 (zero border).
    w_sb:   SBUF [ic, 9*oc] bf16 (tap-major, oc-minor).
    writer(g0, gi, ps): consumes psum [oc, gi*256] for an image group.
    """
    gsz = 2 if nimg >= 2 else 1
    for g0 in range(0, nimg, gsz):
        gi = min(gsz, nimg - g0)
        ps = ps_pool.tile([oc, gi * HW], F32, name=f"ps_conv_{nc.next_id()}",
                          tag="m")
        for t, (dy, dx) in enumerate(TAPS):
            nc.tensor.matmul(ps, w_sb[:, t * oc:(t + 1) * oc],
                             zpad_v[:, g0:g0 + gi, dy:dy + 16, dx:dx + 16],
                             start=(t == 0), stop=(t == 8))
        writer(g0, gi, ps)


def _build():
    nc = bacc.Bacc("TRN2", target_bir_lowering=False)

    d_x0 = nc.dram_tensor("x0", [BL, CIN, 16, 16], F32, kind="ExternalInput")
    d_conn = nc.dram_tensor("conn", [NND, NND], F32, kind="ExternalInput")
    d_cwin = nc.dram_tensor("cwin", [CIN, 9 * HD], F32, kind="ExternalInput")
    d_cbin = nc.dram_tensor("cbin", [HD, 1], F32, kind="ExternalInput")
    d_gw = nc.dram_tensor("gw", [NND, TD_C, 9 * 2 * HD], F32, kind="ExternalInput")
    d_gb = nc.dram_tensor("gb", [NND, 2 * HD, 1], F32, kind="ExternalInput")
    d_cw = nc.dram_tensor("cw", [NND, TD_C, 9 * HD], F32, kind="ExternalInput")
    d_cb = nc.dram_tensor("cb", [NND, HD, 1], F32, kind="ExternalInput")
    d_buw = nc.dram_tensor("buw", [NND, 128, 4 * JBU], F32, kind="ExternalInput")
    d_bub = nc.dram_tensor("bub", [NND, CIN, HW], F32, kind="ExternalInput")
    d_tdw = nc.dram_tensor("tdw", [NND - 1, 8, 128, 4 * JTD], F32, kind="ExternalInput")
    d_tdb = nc.dram_tensor("tdb", [NND - 1, TD_C, HW], F32, kind="ExternalInput")
    d_f1w = nc.dram_tensor("f1w", [128, 32 * 100], F32, kind="ExternalInput")
    d_f1b = nc.dram_tensor("f1b", [100, 1], F32, kind="ExternalInput")
    d_f2w = nc.dram_tensor("f2w", [100, 10], F32, kind="ExternalInput")
    d_f2b = nc.dram_tensor("f2b", [10, 1], F32, kind="ExternalInput")
    d_out = nc.dram_tensor("outT", [10, BL], F32, kind="ExternalOutput")

    rg = [list(range(R))]

    with tile.TileContext(nc) as tc:
        with (
            tc.tile_pool(name="const", bufs=1) as cp,
            tc.tile_pool(name="work", bufs=1) as wp,
            tc.tile_pool(name="sbu", bufs=2) as sbu,
            tc.tile_pool(name="std", bufs=6) as std,
            tc.tile_pool(name="psbu", bufs=1, space="PSUM") as ps_bu,
            tc.tile_pool(name="psgv", bufs=1, space="PSUM") as ps_gv,
            tc.tile_pool(name="psm", bufs=2, space="PSUM") as ps_m,
            tc.tile_pool(name="dram", bufs=1, space="DRAM") as dp,
        ):
            # ---------------- Phase 0: constants --------------------------
            ident = cp.tile([128, 128], F32, name="ident")
            make_identity(nc, ident)

            ones1 = cp.tile([1, 128], BF16, name="ones1")
            nc.gpsimd.memset(ones1, 1.0)
            conn_row = cp.tile([1, 16], BF16, name="conn_row")
            nc.gpsimd.dma_start(out=conn_row, in_=d_conn[:, :])
            ps_cb = ps_m.tile([128, 16], F32, name="ps_cb", tag="m")
            nc.tensor.matmul(ps_cb, ones1, conn_row, start=True, stop=True)
            conn_bc = cp.tile([128, 16], F32, name="conn_bc")
            nc.vector.tensor_copy(conn_bc, ps_cb)

            cwin = cp.tile([CIN, 9 * HD], BF16, name="cwin")
            nc.gpsimd.dma_start(out=cwin, in_=d_cwin[:, :])
            cbin = cp.tile([HD, 1], F32, name="cbin")
            nc.sync.dma_start(out=cbin, in_=d_cbin[:, :])
            gw_sb, cw_sb, gb_sb, cb_sb, bub_sb = [], [], [], [], []
            for n in range(NND):
                t = cp.tile([TD_C, 9 * 2 * HD], BF16, name=f"gw{n}")
                nc.gpsimd.dma_start(out=t, in_=d_gw[n])
                gw_sb.append(t)
                t = cp.tile([TD_C, 9 * HD], BF16, name=f"cw{n}")
                nc.gpsimd.dma_start(out=t, in_=d_cw[n])
                cw_sb.append(t)
                t = cp.tile([2 * HD, 1], F32, name=f"gb{n}")
                nc.sync.dma_start(out=t, in_=d_gb[n])
                gb_sb.append(t)
                t = cp.tile([HD, 1], F32, name=f"cb{n}")
                nc.sync.dma_start(out=t, in_=d_cb[n])
                cb_sb.append(t)
                t = cp.tile([CIN, HW], F32, name=f"bub{n}")
                nc.sync.dma_start(out=t, in_=d_bub[n])
                bub_sb.append(t)
            # td biases, split top (8ch) / bottom (16ch) so all compute APs
            # stay at partition base 0
            tdbt_sb, tdbb_sb = [], []
            for n in range(NND - 1):
                t = cp.tile([CIN, HW], F32, name=f"tdbt{n}")
                nc.sync.dma_start(out=t, in_=d_tdb[n, 0:CIN, :])
                tdbt_sb.append(t)
                t = cp.tile([HD, HW], F32, name=f"tdbb{n}")
                nc.sync.dma_start(out=t, in_=d_tdb[n, CIN:TD_C, :])
                tdbb_sb.append(t)

            # padded conv-input buffers (borders stay zero forever)
            zpad = cp.tile([TD_C, BL * 18 * 18], BF16, name="zpad")
            z2pad = cp.tile([TD_C, BL * 18 * 18], BF16, name="z2pad")
            zpad1 = cp.tile([TD_C, 18 * 18], BF16, name="zpad1")
            xpad = cp.tile([CIN, BL * 18 * 18], BF16, name="xpad")
            nc.gpsimd.memset(zpad, 0.0)
            nc.gpsimd.memset(z2pad, 0.0)
            nc.gpsimd.memset(zpad1, 0.0)
            nc.gpsimd.memset(xpad, 0.0)
            zpad_v = zpad.rearrange("c (b y x) -> c b y x", b=BL, y=18, x=18)
            z2pad_v = z2pad.rearrange("c (b y x) -> c b y x", b=BL, y=18, x=18)
            zpad1_v = zpad1.rearrange("c (b y x) -> c b y x", b=1, y=18, x=18)
            xpad_v = xpad.rearrange("c (b y x) -> c b y x", b=BL, y=18, x=18)

            # ---------------- Phase 1: conv_in ----------------------------
            x0_sb = cp.tile([CIN, BL * HW], F32, name="x0_sb")
            nc.sync.dma_start(
                out=x0_sb.rearrange("c (b s) -> c b s", b=BL),
                in_=d_x0[:, :, :, :].rearrange("b c y x -> c b (y x)"))
            nc.vector.tensor_copy(
                xpad_v[:, :, 1:17, 1:17],
                x0_sb.rearrange("c (b y x) -> c b y x", b=BL, y=16))
            cur_cv = cp.tile([HD, BL * HW], F32, name="cur_cv")
            cur_cv_v = cur_cv.rearrange("c (b s) -> c b s", b=BL)

            def _win(g0, gi, ps):
                nc.vector.tensor_scalar(
                    cur_cv_v[:, g0:g0 + gi, :].rearrange("c b s -> c (b s)"),
                    ps, cbin, None, op0=AOP.add)
            _conv9(nc, ps_m, _win, xpad_v, cwin, HD, BL)

            bnc_cur = dp.tile([R, BL, KS], F32, name="bnc_cur")
            for ch in range(R):
                nc.sync.dma_start(
                    out=bnc_cur[ch].rearrange("b (cc s) -> cc b s", cc=2),
                    in_=cur_cv_v[2 * ch:2 * ch + 2])
            a2a_cur = dp.tile([R, BL, KS], F32, name="a2a_cur")
            nc.gpsimd.collective_compute(
                "AllToAll", AOP.bypass, replica_groups=rg,
                ins=[bnc_cur.opt()], outs=[a2a_cur.opt()])

            # mod for node0 rep0: sigmoid(td_b[0]); h=0 so only top needed
            modb0_t = cp.tile([CIN, HW], F32, name="modb0_t")
            nc.scalar.activation(modb0_t, tdbt_sb[0], ACT.Sigmoid)

            # ---------------- Phase 2: rep-0 constant cells (nodes 1..3) --
            s0c = [None] * NND
            for n in range(1, NND):
                modt = cp.tile([CIN, HW], F32, name=f"modct{n}")
                if n < NND - 1:
                    nc.scalar.activation(modt, tdbt_sb[n], ACT.Sigmoid)
                else:
                    nc.gpsimd.memset(modt, 0.5)
                nc.vector.tensor_mul(
                    zpad1_v[0:CIN, 0, 1:17, 1:17],
                    bub_sb[n].rearrange("c (y x) -> c y x", y=16),
                    modt.rearrange("c (y x) -> c y x", y=16))
                ru = cp.tile([2 * HD, HW], F32, name=f"ru0c{n}")

                def _wg(g0, gi, ps, n=n, ru=ru):
                    nc.scalar.activation(ru, ps, ACT.Sigmoid, bias=gb_sb[n])
                _conv9(nc, ps_m, _wg, zpad1_v, gw_sb[n], 2 * HD, 1)
                u = cp.tile([HD, HW], F32, name=f"u0c{n}")
                nc.sync.dma_start(out=u, in_=ru[HD:2 * HD])
                cand = cp.tile([HD, HW], F32, name=f"cand0c{n}")

                def _wc(g0, gi, ps, n=n, cand=cand):
                    nc.scalar.activation(cand, ps, ACT.Tanh, bias=cb_sb[n])
                _conv9(nc, ps_m, _wc, zpad1_v, cw_sb[n], HD, 1)
                s = cp.tile([HD, HW], F32, name=f"s0c{n}")
                nc.vector.tensor_mul(s, u, cand)
                s0c[n] = s
                nc.gpsimd.memset(zpad1_v[0:CIN, 0, 1:17, 1:17], 0.0)

            # GEMV stationary vectors: vT[n][:, h*16+c] = conn[n+1,n]*s0c[n+1]
            vT = []
            for n in range(NND - 1):
                v = cp.tile([128, 32], BF16, name=f"vT{n}")
                for h in range(2):
                    pst = ps_m.tile([128, HD], F32, name=f"ps_vt{n}{h}",
                                    tag="m")
                    nc.tensor.transpose(
                        pst, s0c[n + 1][:, h * 128:(h + 1) * 128],
                        ident[0:HD, 0:HD])
                    nc.vector.tensor_scalar(
                        v[:, h * HD:(h + 1) * HD], pst,
                        conn_bc[:, (n + 1) * 4 + n:(n + 1) * 4 + n + 1],
                        None, op0=AOP.mult)
                vT.append(v)

            # ---------------- pipeline helpers ----------------------------
            def act_transpose(src_dram, conn_idx, name):
                abm = wp.tile([B, KS], F32, name=f"abm_{name}", tag="abm",
                              bufs=2)
                nc.sync.dma_start(
                    out=abm, in_=src_dram.rearrange("r b s -> (r b) s"))
                at = wp.tile([128, 4 * B], BF16, name=f"actT_{name}",
                             tag="actT", bufs=2)
                for kb in range(4):
                    pst = ps_m.tile([128, B], F32, name=f"ps_at_{name}{kb}",
                                    tag="m")
                    nc.tensor.transpose(
                        pst, abm[:, kb * 128:(kb + 1) * 128], ident[0:B, 0:B])
                    if conn_idx is None:
                        nc.vector.tensor_copy(at[:, kb * B:(kb + 1) * B], pst)
                    else:
                        nc.vector.tensor_scalar(
                            at[:, kb * B:(kb + 1) * B], pst,
                            conn_bc[:, conn_idx:conn_idx + 1],
                            None, op0=AOP.mult)
                return at

            def buw_load(n, name):
                wt = sbu.tile([128, 4 * JBU], BF16, name=f"buw_{name}",
                              tag="buw")
                nc.gpsimd.dma_start(out=wt, in_=d_buw[n])
                return wt

            def bu_gemm(n, wt, actT, name):
                ps = ps_bu.tile([B, JBU], F32, name=f"ps_bu_{name}", tag="bu")
                for kb in range(4):
                    for j in range(4):
                        nc.tensor.matmul(
                            ps[:, j * 512:(j + 1) * 512],
                            actT[:, kb * B:(kb + 1) * B],
                            wt[:, kb * JBU + j * 512:kb * JBU + (j + 1) * 512],
                            start=(kb == 0), stop=(kb == 3))
                part = wp.tile([B, JBU], F32, name=f"bupart_{name}",
                               tag="bupart")
                nc.vector.tensor_copy(part, ps)
                bnc = dp.tile([B, JBU], F32, name=f"bnc_bu_{name}")
                nc.sync.dma_start(out=bnc, in_=part)
                rs = dp.tile([BL, JBU], F32, name=f"rs_bu_{name}")
                nc.gpsimd.collective_compute(
                    "ReduceScatter", AOP.add, replica_groups=rg,
                    ins=[bnc.opt()], outs=[rs.opt()])
                bu_cv = wp.tile([CIN, BL * HW], F32, name=f"bucv_{name}",
                                tag="bucv", bufs=2)
                nc.sync.dma_start(
                    out=bu_cv.rearrange("c (b s) -> c b s", b=BL),
                    in_=rs.rearrange("b (c s) -> c b s", c=CIN))
                return bu_cv

            def td_gemv(n, name):
                """-> (modt [8,HW], modb [16,HW]) for rep-1 cell n."""
                ps = ps_gv.tile([1, JTD], F32, name=f"ps_td_{name}", tag="gv")
                for kb4 in range(8):
                    wt = std.tile([128, 4 * JTD], BF16,
                                  name=f"tdw_{name}{kb4}", tag="tdw")
                    nc.gpsimd.dma_start(out=wt, in_=d_tdw[n, kb4])
                    for ki in range(4):
                        kb = kb4 * 4 + ki
                        col = (kb % 2) * HD + kb // 2
                        lhsT = vT[n][:, col:col + 1]
                        wk = wt[:, ki * JTD:(ki + 1) * JTD]
                        nc.tensor.matmul(ps[:, 0:512], lhsT, wk[:, 0:512],
                                         start=(kb == 0), stop=(kb == 31))
                        nc.tensor.matmul(ps[:, 512:JTD], lhsT,
                                         wk[:, 512:JTD],
                                         start=(kb == 0), stop=(kb == 31))
                tds = wp.tile([1, JTD], F32, name=f"tdsb_{name}", tag="tdsb")
                nc.vector.tensor_copy(tds, ps)
                bnc = dp.tile([1, JTD], F32, name=f"bnc_td_{name}")
                nc.sync.dma_start(out=bnc, in_=tds)
                ag = dp.tile([R, JTD], F32, name=f"ag_td_{name}",
                             addr_space="Shared")
                nc.gpsimd.collective_compute(
                    "AllGather", AOP.bypass, replica_groups=rg,
                    ins=[bnc.opt()], outs=[ag.opt()])
                agv = ag.rearrange("r (a s) -> (r a) s", s=HW)
                tdf_t = wp.tile([CIN, HW], F32, name=f"tdft_{name}",
                                tag="tdft")
                nc.sync.dma_start(out=tdf_t, in_=agv[0:CIN])
                tdf_b = wp.tile([HD, HW], F32, name=f"tdfb_{name}",
                                tag="tdfb")
                nc.sync.dma_start(out=tdf_b, in_=agv[CIN:TD_C])
                nc.vector.tensor_add(tdf_t, tdf_t, tdbt_sb[n])
                nc.vector.tensor_add(tdf_b, tdf_b, tdbb_sb[n])
                modt = cp.tile([CIN, HW], F32, name=f"modt_{name}")
                nc.scalar.activation(modt, tdf_t, ACT.Sigmoid)
                modb = cp.tile([HD, HW], F32, name=f"modb_{name}")
                nc.scalar.activation(modb, tdf_b, ACT.Sigmoid)
                return modt, modb

            def zbot_dma(dst_pad_v, src_v4):
                """DMA [16, BL, 16, 16] into partitions 8:24 of a padded
                conv buffer (DMA is exempt from partition-base rules)."""
                for b in range(BL):
                    nc.sync.dma_start(
                        out=dst_pad_v[CIN:TD_C, b, 1:17, 1:17],
                        in_=src_v4[:, b])

            def cell(n, bu_cv, h, h_const, modt, modb, name):
                """Rep-1 cell for BL local images -> state [HD, BL*HW] f32.

                bu_cv: [CIN, BL*HW] f32 (bias not yet added)
                h: [HD, BL*HW] f32 (batch-var) or [HD, HW] f32 (batch-const)
                modt/modb: [CIN, HW] / [HD, HW] f32, batch-constant
                """
                bu_cv4 = bu_cv.rearrange("c (b y x) -> c b y x", b=BL, y=16)
                mod4t = modt.rearrange("c (y x) -> c y x", y=16)
                modt_bc = mod4t[:, None, :, :].broadcast_to([CIN, BL, 16, 16])
                mod4b = modb.rearrange("c (y x) -> c y x", y=16)
                modb_bc = mod4b[:, None, :, :].broadcast_to([HD, BL, 16, 16])
                bub4 = bub_sb[n].rearrange("c (y x) -> c y x", y=16)
                bub_bc = bub4[:, None, :, :].broadcast_to([CIN, BL, 16, 16])
                # z top = (bu + bias) * mod_top
                bub_t = wp.tile([CIN, BL * HW], F32, name=f"bubt_{name}",
                                tag="bubt")
                bub_tv = bub_t.rearrange("c (b y x) -> c b y x", b=BL, y=16)
                nc.vector.tensor_add(bub_tv, bu_cv4, bub_bc)
                nc.vector.tensor_mul(zpad_v[0:CIN, :, 1:17, 1:17],
                                     bub_tv, modt_bc)
                # z bottom = h * mod_bot, staged base-0 then DMA'd to 8:24
                zbot = wp.tile([HD, BL * HW], BF16, name=f"zbot_{name}",
                               tag="zbot")
                zbot_v = zbot.rearrange("c (b y x) -> c b y x", b=BL, y=16)
                if h_const:
                    hm = wp.tile([HD, HW], F32, name=f"hm_{name}", tag="hm")
                    nc.vector.tensor_mul(hm, h, modb)
                    hm4 = hm.rearrange("c (y x) -> c y x", y=16)
                    hm_bc = hm4[:, None, :, :].broadcast_to([HD, BL, 16, 16])
                    nc.vector.tensor_copy(zbot_v, hm_bc)
                    h_bc = h.rearrange("c (y x) -> c y x", y=16)[
                        :, None, :, :].broadcast_to([HD, BL, 16, 16])
                else:
                    h_v = h.rearrange("c (b y x) -> c b y x", b=BL, y=16)
                    nc.vector.tensor_mul(zbot_v, h_v, modb_bc)
                    h_bc = h_v
                    hm_bc = None
                zbot_dma(zpad_v, zbot_v)
                # gates
                ru = wp.tile([2 * HD, BL * HW], F32, name=f"ru_{name}",
                             tag="ru")
                ru_v = ru.rearrange("c (b s) -> c b s", b=BL)
                ru_v4 = ru.rearrange("c (b y x) -> c b y x", b=BL, y=16)

                def _wg(g0, gi, ps):
                    nc.scalar.activation(
                        ru_v[:, g0:g0 + gi, :].rearrange("c b s -> c (b s)"),
                        ps, ACT.Sigmoid, bias=gb_sb[n])
                _conv9(nc, ps_m, _wg, zpad_v, gw_sb[n], 2 * HD, BL)
                u = wp.tile([HD, BL * HW], F32, name=f"u_{name}", tag="u")
                nc.sync.dma_start(out=u, in_=ru[HD:2 * HD])
                # z2: top copied, bottom = (r*h) * mod_bot
                nc.vector.tensor_copy(z2pad_v[0:CIN, :, 1:17, 1:17],
                                      zpad_v[0:CIN, :, 1:17, 1:17])
                rh = wp.tile([HD, BL * HW], BF16, name=f"rh_{name}", tag="rh")
                rh_v = rh.rearrange("c (b y x) -> c b y x", b=BL, y=16)
                if h_const:
                    nc.vector.tensor_mul(rh_v, ru_v4[0:HD], hm_bc)
                else:
                    rhf = wp.tile([HD, BL * HW], F32, name=f"rhf_{name}",
                                  tag="rhf")
                    rhf_v = rhf.rearrange("c (b y x) -> c b y x", b=BL, y=16)
                    nc.vector.tensor_mul(rhf_v, ru_v4[0:HD], h_bc)
                    nc.vector.tensor_mul(rh_v, rhf_v, modb_bc)
                zbot_dma(z2pad_v, rh_v)
                # cand
                cand = wp.tile([HD, BL * HW], F32, name=f"cand_{name}",
                               tag="cand")
                cand_v = cand.rearrange("c (b s) -> c b s", b=BL)

                def _wc(g0, gi, ps):
                    nc.scalar.activation(
                        cand_v[:, g0:g0 + gi, :].rearrange("c b s -> c (b s)"),
                        ps, ACT.Tanh, bias=cb_sb[n])
                _conv9(nc, ps_m, _wc, z2pad_v, cw_sb[n], HD, BL)
                # state = h + u * (cand - h)
                st = wp.tile([HD, BL * HW], F32, name=f"st_{name}", tag="st",
                             bufs=3)
                st_v = st.rearrange("c (b y x) -> c b y x", b=BL, y=16)
                cand_v4 = cand.rearrange("c (b y x) -> c b y x", b=BL, y=16)
                nc.vector.tensor_sub(st_v, cand_v4, h_bc)
                nc.vector.tensor_mul(st, u, st)
                nc.vector.tensor_add(st_v, st_v, h_bc)
                return st

            def state_a2a(st, name):
                bnc = dp.tile([R, BL, KS], F32, name=f"bnc_st_{name}")
                st_v3 = st.rearrange("c (b s) -> c b s", b=BL)
                for ch in range(R):
                    nc.sync.dma_start(
                        out=bnc[ch].rearrange("b (cc s) -> cc b s", cc=2),
                        in_=st_v3[2 * ch:2 * ch + 2])
                a2a = dp.tile([R, BL, KS], F32, name=f"a2a_st_{name}")
                nc.gpsimd.collective_compute(
                    "AllToAll", AOP.bypass, replica_groups=rg,
                    ins=[bnc.opt()], outs=[a2a.opt()])
                return a2a

            # ---------------- node chain ----------------------------------
            def emit_out_from(ap2d):
                """Early-exit: write outT from whatever we have (top-left)."""
                p = min(10, ap2d.shape[0])
                tmp = wp.tile([10, BL], F32, name="early_out")
                nc.vector.memset(tmp, 0.0)
                nc.vector.tensor_copy(tmp[0:p], ap2d[0:p, 0:BL])
                nc.sync.dma_start(out=d_out[:, :], in_=tmp)

            done = False
            buw0 = buw_load(0, "n0")
            actT0 = act_transpose(a2a_cur, None, "n0")
            if TRUNC == 1:
                emit_out_from(actT0); done = True
            if not done:
                bu0_cv = bu_gemm(0, buw0, actT0, "n0")
                if TRUNC == 2:
                    emit_out_from(bu0_cv); done = True
            if not done:
                mod0_t, mod0_b = td_gemv(0, "n0")
                if TRUNC == 3:
                    emit_out_from(mod0_b); done = True

            if not done:
                # rep0 cell node0: h = 0, mod top = sigmoid(td_b[0]), z2 == z
                ru0 = wp.tile([2 * HD, BL * HW], F32, name="ru0", tag="ru")
                ru0_v = ru0.rearrange("c (b s) -> c b s", b=BL)
                bub0 = wp.tile([CIN, BL * HW], F32, name="bub0", tag="bubt")
                bub0_v4 = bub0.rearrange("c (b y x) -> c b y x", b=BL, y=16)
                bub04 = bub_sb[0].rearrange("c (y x) -> c y x", y=16)
                nc.vector.tensor_add(
                    bub0_v4, bu0_cv.rearrange("c (b y x) -> c b y x", y=16, b=BL),
                    bub04[:, None, :, :].broadcast_to([CIN, BL, 16, 16]))
                mb04 = modb0_t.rearrange("c (y x) -> c y x", y=16)
                nc.vector.tensor_mul(
                    zpad_v[0:CIN, :, 1:17, 1:17], bub0_v4,
                    mb04[:, None, :, :].broadcast_to([CIN, BL, 16, 16]))
                # z bottom stays zero (h = 0; zpad fresh from memset)

                def _wg0(g0, gi, ps):
                    nc.scalar.activation(
                        ru0_v[:, g0:g0 + gi, :].rearrange("c b s -> c (b s)"),
                        ps, ACT.Sigmoid, bias=gb_sb[0])
                _conv9(nc, ps_m, _wg0, zpad_v, gw_sb[0], 2 * HD, BL)
                u0 = wp.tile([HD, BL * HW], F32, name="u0", tag="u")
                nc.sync.dma_start(out=u0, in_=ru0[HD:2 * HD])
                cand0 = wp.tile([HD, BL * HW], F32, name="cand0", tag="cand")
                cand0_v = cand0.rearrange("c (b s) -> c b s", b=BL)

                def _wc0(g0, gi, ps):
                    nc.scalar.activation(
                        cand0_v[:, g0:g0 + gi, :].rearrange("c b s -> c (b s)"),
                        ps, ACT.Tanh, bias=cb_sb[0])
                _conv9(nc, ps_m, _wc0, zpad_v, cw_sb[0], HD, BL)
                s0_r0 = wp.tile([HD, BL * HW], F32, name="s0_r0", tag="s0r0")
                nc.vector.tensor_mul(s0_r0, u0, cand0)

                if TRUNC == 4:
                    emit_out_from(s0_r0)
                if TRUNC > 4:
                    st = cell(0, bu0_cv, s0_r0, False, mod0_t, mod0_b, "c0r1")
                    a2a = state_a2a(st, "n0")
                    if TRUNC == 5:
                        emit_out_from(st)
                if TRUNC > 5:
                    for n in range(1, NND):
                        nm = f"n{n}"
                        wt = buw_load(n, nm)
                        if n < NND - 1:
                            modt, modb = td_gemv(n, nm)
                        else:
                            modt = cp.tile([CIN, HW], F32, name="mod3_t")
                            nc.gpsimd.memset(modt, 0.5)
                            modb = cp.tile([HD, HW], F32, name="mod3_b")
                            nc.gpsimd.memset(modb, 0.5)
                        actT = act_transpose(a2a, (n - 1) * 4 + n, nm)
                        bu_cv = bu_gemm(n, wt, actT, nm)
                        st = cell(n, bu_cv, s0c[n], True, modt, modb,
                                  f"c{n}r1")
                        if n < NND - 1:
                            a2a = state_a2a(st, nm)
                        if TRUNC == 6 + (n - 1):
                            emit_out_from(st)
                            break
                if TRUNC > 8:
                    # ------------- head -----------------------------------
                    f1w = cp.tile([128, 32 * 100], BF16, name="f1w")
                    nc.gpsimd.dma_start(out=f1w, in_=d_f1w[:, :])
                    f1b = cp.tile([100, 1], F32, name="f1b")
                    nc.sync.dma_start(out=f1b, in_=d_f1b[:, :])
                    f2w = cp.tile([100, 10], BF16, name="f2w")
                    nc.gpsimd.dma_start(out=f2w, in_=d_f2w[:, :])
                    f2b = cp.tile([10, 1], F32, name="f2b")
                    nc.sync.dma_start(out=f2b, in_=d_f2b[:, :])

                    s3r = wp.tile([HD, BL * HW], F32, name="s3r", tag="s3r")
                    nc.vector.tensor_scalar(s3r, st, 0.0, None, op0=AOP.max)
                    s3r_v = s3r.rearrange("c (b s) -> c b s", b=BL)
                    s3T = cp.tile([128, 128], BF16, name="s3T")
                    for b in range(BL):
                        for h in range(2):
                            pst = ps_m.tile([128, HD], F32,
                                            name=f"ps_h{b}{h}", tag="m")
                            nc.tensor.transpose(
                                pst, s3r_v[:, b, h * 128:(h + 1) * 128],
                                ident[0:HD, 0:HD])
                            nc.vector.tensor_copy(
                                s3T[:, b * 32 + h * HD:b * 32 + (h + 1) * HD],
                                pst)
                    ps1 = ps_m.tile([100, BL], F32, name="ps_fc1", tag="m")
                    for kb in range(32):
                        c_, h_ = kb // 2, kb % 2
                        nc.tensor.matmul(
                            ps1, f1w[:, kb * 100:(kb + 1) * 100],
                            s3T[:, h_ * HD + c_:128:32],
                            start=(kb == 0), stop=(kb == 31))
                    h1 = wp.tile([100, BL], BF16, name="h1")
                    nc.scalar.activation(h1, ps1, ACT.Relu, bias=f1b)
                    ps2 = ps_m.tile([10, BL], F32, name="ps_fc2", tag="m")
                    nc.tensor.matmul(ps2, f2w, h1, start=True, stop=True)
                    outT = wp.tile([10, BL], F32, name="outT_sb")
                    nc.vector.tensor_scalar(outT, ps2, f2b, None, op0=AOP.add)
                    nc.sync.dma_start(out=d_out[:, :], in_=outT)

    nc.finalize()
    return nc


def _get_nc():
    global _CACHED_NC
    if _CACHED_NC is None:
        _CACHED_NC = _build()
    return _CACHED_NC


def _prep_inputs(inputs):
    f = lambda a: np.ascontiguousarray(np.asarray(a), dtype=np.float32)
    x = f(inputs["x"])
    conn = f(inputs["conn"])
    cwin = f(inputs["conv_in_w"]).transpose(1, 2, 3, 0).reshape(CIN, 9 * HD)
    cbin = f(inputs["conv_in_b"]).reshape(HD, 1)
    gw = f(inputs["gate_w"]).transpose(0, 2, 3, 4, 1).reshape(
        NND, TD_C, 9 * 2 * HD)
    gb = f(inputs["gate_b"]).reshape(NND, 2 * HD, 1)
    cw = f(inputs["cand_w"]).transpose(0, 2, 3, 4, 1).reshape(NND, TD_C, 9 * HD)
    cb = f(inputs["cand_b"]).reshape(NND, HD, 1)
    buwT = f(inputs["bu_w"]).transpose(0, 2, 1)          # (4, 4096, 2048)
    bub = f(inputs["bu_b"]).reshape(NND, CIN, HW)
    tdwT = f(inputs["td_w"]).transpose(0, 2, 1)
    tdb = f(inputs["td_b"]).reshape(NND - 1, TD_C, HW)
    f1w = np.ascontiguousarray(
        f(inputs["fc1_w"]).T.reshape(32, 128, 100).transpose(1, 0, 2)
        .reshape(128, 3200))
    f1b = f(inputs["fc1_b"]).reshape(100, 1)
    f2w = np.ascontiguousarray(f(inputs["fc2_w"]).T)
    f2b = f(inputs["fc2_b"]).reshape(10, 1)

    in_maps = []
    for c in range(R):
        in_maps.append({
            "x0": np.ascontiguousarray(x[c * BL:(c + 1) * BL, 0]),
            "conn": conn,
            "cwin": cwin, "cbin": cbin,
            "gw": gw, "gb": gb, "cw": cw, "cb": cb,
            "buw": np.ascontiguousarray(
                buwT[:, c * KS:(c + 1) * KS, :]
                .reshape(NND, 4, 128, JBU).transpose(0, 2, 1, 3)
                .reshape(NND, 128, 4 * JBU)),
            "bub": bub,
            "tdw": np.ascontiguousarray(
                tdwT[:, :, c * JTD:(c + 1) * JTD]
                .reshape(NND - 1, 8, 4, 128, JTD).transpose(0, 1, 3, 2, 4)
                .reshape(NND - 1, 8, 128, 4 * JTD)),
            "tdb": tdb,
            "f1w": f1w, "f1b": f1b, "f2w": f2w, "f2b": f2b,
        })
    return in_maps


def run(inputs, trace=False):
    nc = _get_nc()
    in_maps = _prep_inputs(inputs)
    res = run_bass_kernel_spmd(nc, in_maps, core_ids=list(range(R)),
                               trace=trace)
    out = np.concatenate([r["outT"].T for r in res.results], axis=0)
    return out.astype(np.float32), res


def kernel(**inputs):
    out, _ = run(inputs, trace=False)
    return out


if __name__ == "__main__":
    _build()
    print("build OK")

